# Initial kernel scaffold
#
"""Bahdanau attention Trainium2 kernel.

Full-input contract: kernel(**inputs) takes the complete unsharded arrays
(B=8, Td=64, Te=1024, D=256, U=128), shards batch-parallel across 8
NeuronCores (one batch element per core), runs a Bass/Tile kernel via
bass_utils.run_bass_kernel_spmd, and returns the full [8, 64, 256] output.

Per-core algorithm (all shapes hardcoded):
  w_encT [U,Te] = W1.T @ encT + b1   (U on partitions)
  w_decT [U,Td] = W2.T @ decT + b2
  for t in 0..63:
      tanh_t [U,Te] = tanh(w_encT + w_decT[:,t])      # ACT, bias fused
      score[t, :]  += V . tanh_t                      # PE, banded-C trick
  attn = softmax(score, axis=-1)  -> fold 1/sum into output scale
  out [Td,D] = (exp(score - max) @ enc) * (1/sum)

The V-contraction uses a banded weight matrix C [128, 127] with V stored in
column 63: lhsT = C[:, 63-t : 127-t] puts V at output partition t, so 64
accumulating matmuls build the entire score [64, 1024] tile in PSUM with no
per-row evacuation.  bV is dropped entirely (softmax is shift-invariant).
"""

import numpy as np

import concourse.bass as bass
import concourse.tile as tile
from concourse import mybir
from concourse import bass_utils
from concourse.masks import make_identity

B, TD, TE, D, U = 8, 64, 1024, 256, 128
P = 128
NS = TE // P  # 8 encoder-time chunks
ND = D // P   # 2 feature chunks
F32 = mybir.dt.float32
BF16 = mybir.dt.bfloat16
AF = mybir.ActivationFunctionType

# group size for the DVE-add + big-ACT tanh path (amortizes the ~352-cycle
# per-instruction ACT overhead; 0 disables grouping -> per-t fused bias)
ZGROUP = 8


def _build_kernel(tc: tile.TileContext, ins: dict, outs: dict):
    nc = tc.nc
    enc, dec, W1, b1, W2, b2, V = (
        ins["enc"], ins["dec"], ins["W1"], ins["b1"], ins["W2"], ins["b2"], ins["V"],
    )
    out = outs["out"]

    singles = tc.alloc_tile_pool(name="singles", bufs=1)
    psum_mm = tc.alloc_tile_pool(name="psum_mm", bufs=2, space="PSUM")
    psum_tr = tc.alloc_tile_pool(name="psum_tr", bufs=2, space="PSUM")
    psum_sc = tc.alloc_tile_pool(name="psum_sc", bufs=1, space="PSUM")
    tanh_pool = tc.alloc_tile_pool(name="tanh_pool", bufs=2)
    z_pool = tc.alloc_tile_pool(name="z_pool", bufs=2)
    small = tc.alloc_tile_pool(name="small", bufs=2)

    ident = singles.tile([P, P], F32)
    make_identity(nc, ident)

    # ---- loads ----
    enc_sb = singles.tile([P, NS, D], F32)
    for k in range(NS):
        nc.sync.dma_start(enc_sb[:, k], enc[k * P:(k + 1) * P, :])
    dec_sb = singles.tile([TD, D], F32)
    nc.sync.dma_start(dec_sb, dec)
    w1_sb = singles.tile([P, ND, U], F32)
    w2_sb = singles.tile([P, ND, U], F32)
    for k in range(ND):
        nc.sync.dma_start(w1_sb[:, k], W1[k * P:(k + 1) * P, :])
        nc.sync.dma_start(w2_sb[:, k], W2[k * P:(k + 1) * P, :])
    b1_sb = singles.tile([U, 1], F32)
    nc.sync.dma_start(b1_sb, b1)
    b2_sb = singles.tile([U, 1], F32)
    nc.sync.dma_start(b2_sb, b2)
    v_sb = singles.tile([U, 1], F32)
    nc.sync.dma_start(v_sb, V)

    # banded V matrix: C[:, 63] = V, zeros elsewhere; C[:, 63-t : 127-t]
    # is V-at-column-t
    c_band = singles.tile([U, 2 * TD - 1], BF16)
    nc.vector.memset(c_band, 0.0)
    nc.vector.tensor_copy(c_band[:, TD - 1:TD], v_sb)

    # ---- encT / decT via PE transpose ----
    encT = singles.tile([P, ND, TE], F32)
    for k in range(NS):
        for d in range(ND):
            pst = psum_tr.tile([P, P], F32, tag="tr")
            nc.tensor.transpose(pst, enc_sb[:, k, d * P:(d + 1) * P], ident)
            nc.vector.tensor_copy(encT[:, d, k * P:(k + 1) * P], pst)
    decT = singles.tile([P, ND, TD], F32)
    for d in range(ND):
        pst = psum_tr.tile([P, TD], F32, tag="tr")
        nc.tensor.transpose(pst, dec_sb[:, d * P:(d + 1) * P], ident[:TD, :TD])
        nc.vector.tensor_copy(decT[:, d], pst)

    # ---- w_encT = W1.T @ encT + b1 ; w_decT = W2.T @ decT + b2 ----
    w_encT = singles.tile([U, TE], F32)
    for n in range(2):
        ps = psum_mm.tile([U, 512], F32, tag="wenc")
        for k in range(ND):
            nc.tensor.matmul(ps, w1_sb[:, k], encT[:, k, n * 512:(n + 1) * 512],
                             start=(k == 0), stop=(k == ND - 1))
        nc.vector.tensor_scalar_add(w_encT[:, n * 512:(n + 1) * 512], ps, b1_sb)
    w_decT = singles.tile([U, TD], F32)
    psd = psum_mm.tile([U, TD], F32, tag="wdec")
    for k in range(ND):
        nc.tensor.matmul(psd, w2_sb[:, k], decT[:, k],
                         start=(k == 0), stop=(k == ND - 1))
    nc.vector.tensor_scalar_add(w_decT, psd, b2_sb)

    # ---- main loop: tanh + V-contraction into score PSUM [64, 1024] ----
    score_ps = psum_sc.tile([TD, TE], F32)

    def v_matmuls(t, th, col0):
        for n in range(2):
            nc.tensor.matmul(
                score_ps[:, n * 512:(n + 1) * 512],
                c_band[:, TD - 1 - t:2 * TD - 1 - t],
                th[:, col0 + n * 512:col0 + (n + 1) * 512],
                start=(t == 0), stop=(t == TD - 1),
            )

    if ZGROUP <= 1:
        for t in range(TD):
            th = tanh_pool.tile([U, TE], BF16, tag="tanh")
            nc.scalar.activation(th, w_encT, AF.Tanh,
                                 bias=w_decT[:, t:t + 1], scale=1.0)
            v_matmuls(t, th, 0)
    else:
        G = ZGROUP
        for g in range(TD // G):
            z = z_pool.tile([U, G * TE], F32, tag="z")
            for j in range(G):
                t = g * G + j
                nc.vector.tensor_scalar_add(
                    z[:, j * TE:(j + 1) * TE], w_encT, w_decT[:, t:t + 1])
            th = tanh_pool.tile([U, G * TE], BF16, tag="tanh")
            nc.scalar.activation(th, z, AF.Tanh)
            for j in range(G):
                v_matmuls(g * G + j, th, j * TE)

    # ---- softmax (normalization folded into output scale) ----
    negmax = small.tile([TD, 1], F32, tag="negmax")
    nc.vector.tensor_reduce(negmax, score_ps, mybir.AxisListType.X,
                            mybir.AluOpType.max, negate=True)
    E = singles.tile([TD, TE], F32)
    nc.scalar.activation(E, score_ps, AF.Exp, bias=negmax, scale=1.0)
    ssum = small.tile([TD, 1], F32, tag="ssum")
    nc.vector.tensor_reduce(ssum, E, mybir.AxisListType.X, mybir.AluOpType.add)
    rsum = small.tile([TD, 1], F32, tag="rsum")
    nc.vector.reciprocal(rsum, ssum)

    # ---- context = (E @ enc) * rsum ----
    ET = singles.tile([P, NS, TD], F32)
    for k in range(NS):
        pst = psum_tr.tile([P, TD], F32, tag="tr")
        nc.tensor.transpose(pst, E[:, k * P:(k + 1) * P], ident[:TD, :TD])
        nc.vector.tensor_copy(ET[:, k], pst)
    ctx_ps = psum_mm.tile([TD, D], F32, tag="ctx")
    for k in range(NS):
        nc.tensor.matmul(ctx_ps, ET[:, k], enc_sb[:, k],
                         start=(k == 0), stop=(k == NS - 1))
    out_sb = singles.tile([TD, D], F32)
    nc.vector.tensor_scalar_mul(out_sb, ctx_ps, rsum)
    nc.sync.dma_start(out, out_sb)


_CACHE = {}


def _get_nc():
    if "nc" in _CACHE:
        return _CACHE["nc"]
    nc = bass.Bass("TRN2", target_bir_lowering=False, debug=False,
                   enable_asserts=True, num_devices=B)
    ins = {
        "enc": nc.dram_tensor("enc", [TE, D], F32, kind="ExternalInput").ap(),
        "dec": nc.dram_tensor("dec", [TD, D], F32, kind="ExternalInput").ap(),
        "W1": nc.dram_tensor("W1", [D, U], F32, kind="ExternalInput").ap(),
        "b1": nc.dram_tensor("b1", [U, 1], F32, kind="ExternalInput").ap(),
        "W2": nc.dram_tensor("W2", [D, U], F32, kind="ExternalInput").ap(),
        "b2": nc.dram_tensor("b2", [U, 1], F32, kind="ExternalInput").ap(),
        "V": nc.dram_tensor("V", [U, 1], F32, kind="ExternalInput").ap(),
    }
    outs = {"out": nc.dram_tensor("out", [TD, D], F32, kind="ExternalOutput").ap()}
    with tile.TileContext(nc) as tc:
        _build_kernel(tc, ins, outs)
    _CACHE["nc"] = nc
    return nc


def _in_maps(decoder_output, encoder_output, W1, b1, W2, b2, V):
    f = np.float32
    maps = []
    for b in range(B):
        maps.append({
            "enc": np.ascontiguousarray(encoder_output[b], dtype=f),
            "dec": np.ascontiguousarray(decoder_output[b], dtype=f),
            "W1": np.ascontiguousarray(W1, dtype=f),
            "b1": np.ascontiguousarray(np.asarray(b1, dtype=f).reshape(U, 1)),
            "W2": np.ascontiguousarray(W2, dtype=f),
            "b2": np.ascontiguousarray(np.asarray(b2, dtype=f).reshape(U, 1)),
            "V": np.ascontiguousarray(np.asarray(V, dtype=f).reshape(U, 1)),
        })
    return maps


def run(decoder_output, encoder_output, W1, b1, W2, b2, V, bV=None, *,
        trace=False, **trace_kwargs):
    nc = _get_nc()
    maps = _in_maps(decoder_output, encoder_output, W1, b1, W2, b2, V)
    res = bass_utils.run_bass_kernel_spmd(
        nc, maps, core_ids=list(range(B)), trace=trace, **trace_kwargs)
    out = np.stack([r["out"] for r in res.results], axis=0)
    return out.astype(np.float32), res


def kernel(decoder_output, encoder_output, W1, b1, W2, b2, V, bV=None):
    out, _ = run(decoder_output, encoder_output, W1, b1, W2, b2, V, bV)
    return out


# revision 7
# speedup vs baseline: 1.3956x; 1.3956x over previous
"""Bahdanau attention Trainium2 kernel.

Full-input contract: kernel(**inputs) takes the complete unsharded arrays
(B=8, Td=64, Te=1024, D=256, U=128), shards batch-parallel across 8
NeuronCores (one batch element per core), runs a Bass/Tile kernel via
bass_utils.run_bass_kernel_spmd, and returns the full [8, 64, 256] output.

Per-core algorithm (all shapes hardcoded):
  w_encT [U,Te] = W1.T @ encT + b1   (U on partitions)
  w_decT [U,Td] = W2.T @ decT + b2
  for t in 0..63:
      tanh_t [U,Te] = tanh(w_encT + w_decT[:,t])      # ACT, bias fused
      score[t, :]  += V . tanh_t                      # PE, banded-C trick
  attn = softmax(score, axis=-1)  -> fold 1/sum into output scale
  out [Td,D] = (exp(score - max) @ enc) * (1/sum)

The V-contraction uses a banded weight matrix C [128, 127] with V stored in
column 63: lhsT = C[:, 63-t : 127-t] puts V at output partition t, so 64
accumulating matmuls build the entire score [64, 1024] tile in PSUM with no
per-row evacuation.  bV is dropped entirely (softmax is shift-invariant).
"""

import numpy as np

import concourse.bass as bass
import concourse.tile as tile
from concourse import bacc
from concourse import mybir
from concourse import bass_utils
from concourse.masks import make_identity

B, TD, TE, D, U = 8, 64, 1024, 256, 128
P = 128
NS = TE // P  # 8 encoder-time chunks
ND = D // P   # 2 feature chunks
F32 = mybir.dt.float32
BF16 = mybir.dt.float16  # fp16: same PE rate as bf16, 4x finer mantissa; tanh in [-1,1] fits
AF = mybir.ActivationFunctionType

# group size for the DVE-add + big-ACT tanh path (amortizes the ~352-cycle
# per-instruction ACT overhead; 0 disables grouping -> per-t fused bias)
ZGROUP = 8


def _make_pools(ctx, tc: tile.TileContext):
    return dict(
        singles=ctx.enter_context(tc.tile_pool(name="singles", bufs=1)),
        psum_mm=ctx.enter_context(tc.tile_pool(name="psum_mm", bufs=2, space="PSUM")),
        psum_tr=ctx.enter_context(tc.tile_pool(name="psum_tr", bufs=2, space="PSUM")),
        psum_sc=ctx.enter_context(tc.tile_pool(name="psum_sc", bufs=1, space="PSUM")),
        tanh_pool=ctx.enter_context(tc.tile_pool(name="tanh_pool", bufs=2)),
        z_pool=ctx.enter_context(tc.tile_pool(name="z_pool", bufs=2)),
        small=ctx.enter_context(tc.tile_pool(name="small", bufs=2)),
    )


def _build_kernel(tc: tile.TileContext, pools: dict, ins: dict, outs: dict):
    nc = tc.nc
    enc, dec, W1, b1, W2, b2, V = (
        ins["enc"], ins["dec"], ins["W1"], ins["b1"], ins["W2"], ins["b2"], ins["V"],
    )
    out = outs["out"]

    singles = pools["singles"]
    psum_mm = pools["psum_mm"]
    psum_tr = pools["psum_tr"]
    psum_sc = pools["psum_sc"]
    tanh_pool = pools["tanh_pool"]
    z_pool = pools["z_pool"]
    small = pools["small"]

    ident = singles.tile([P, P], F32)
    make_identity(nc, ident)

    # ---- loads ----
    enc_sb = singles.tile([P, NS, D], F32)
    for k in range(NS):
        nc.sync.dma_start(enc_sb[:, k], enc[k * P:(k + 1) * P, :])
    dec_sb = singles.tile([TD, D], F32)
    nc.sync.dma_start(dec_sb, dec)
    w1_sb = singles.tile([P, ND, U], F32)
    w2_sb = singles.tile([P, ND, U], F32)
    for k in range(ND):
        nc.sync.dma_start(w1_sb[:, k], W1[k * P:(k + 1) * P, :])
        nc.sync.dma_start(w2_sb[:, k], W2[k * P:(k + 1) * P, :])
    b1_sb = singles.tile([U, 1], F32)
    nc.sync.dma_start(b1_sb, b1)
    b2_sb = singles.tile([U, 1], F32)
    nc.sync.dma_start(b2_sb, b2)
    v_sb = singles.tile([U, 1], F32)
    nc.sync.dma_start(v_sb, V)

    # banded V matrix: C[:, 63] = V, zeros elsewhere; C[:, 63-t : 127-t]
    # is V-at-column-t
    c_band = singles.tile([U, 2 * TD - 1], BF16)
    nc.vector.memset(c_band, 0.0)
    nc.vector.tensor_copy(c_band[:, TD - 1:TD], v_sb)

    # ---- encT / decT via PE transpose ----
    encT = singles.tile([P, ND, TE], F32)
    for k in range(NS):
        for d in range(ND):
            pst = psum_tr.tile([P, P], F32, tag="tr")
            nc.tensor.transpose(pst, enc_sb[:, k, d * P:(d + 1) * P], ident)
            nc.vector.tensor_copy(encT[:, d, k * P:(k + 1) * P], pst)
    decT = singles.tile([P, ND, TD], F32)
    for d in range(ND):
        pst = psum_tr.tile([P, TD], F32, tag="tr")
        nc.tensor.transpose(pst, dec_sb[:, d * P:(d + 1) * P], ident[:TD, :TD])
        nc.vector.tensor_copy(decT[:, d], pst)

    # ---- w_encT = W1.T @ encT + b1 ; w_decT = W2.T @ decT + b2 ----
    w_encT = singles.tile([U, TE], F32)
    for n in range(2):
        ps = psum_mm.tile([U, 512], F32, tag="mm")
        for k in range(ND):
            nc.tensor.matmul(ps, w1_sb[:, k], encT[:, k, n * 512:(n + 1) * 512],
                             start=(k == 0), stop=(k == ND - 1))
        nc.vector.tensor_scalar_add(w_encT[:, n * 512:(n + 1) * 512], ps, b1_sb)
    w_decT = singles.tile([U, TD], F32)
    psd = psum_mm.tile([U, TD], F32, tag="mm")
    for k in range(ND):
        nc.tensor.matmul(psd, w2_sb[:, k], decT[:, k],
                         start=(k == 0), stop=(k == ND - 1))
    nc.vector.tensor_scalar_add(w_decT, psd, b2_sb)

    # ---- main loop: tanh + V-contraction into score PSUM [64, 1024] ----
    score_ps = psum_sc.tile([TD, TE], F32)

    def v_matmuls(t, th, col0):
        for n in range(2):
            nc.tensor.matmul(
                score_ps[:, n * 512:(n + 1) * 512],
                c_band[:, TD - 1 - t:2 * TD - 1 - t],
                th[:, col0 + n * 512:col0 + (n + 1) * 512],
                start=(t == 0), stop=(t == TD - 1),
            )

    if ZGROUP <= 1:
        for t in range(TD):
            th = tanh_pool.tile([U, TE], BF16, tag="tanh")
            nc.scalar.activation(th, w_encT, AF.Tanh,
                                 bias=w_decT[:, t:t + 1], scale=1.0)
            v_matmuls(t, th, 0)
    else:
        G = ZGROUP
        for g in range(TD // G):
            z = z_pool.tile([U, G * TE], F32, tag="z")
            for j in range(G):
                t = g * G + j
                nc.vector.tensor_scalar_add(
                    z[:, j * TE:(j + 1) * TE], w_encT, w_decT[:, t:t + 1])
            th = tanh_pool.tile([U, G * TE], BF16, tag="tanh")
            nc.scalar.activation(th, z, AF.Tanh)
            for j in range(G):
                v_matmuls(g * G + j, th, j * TE)

    # ---- softmax (normalization folded into output scale) ----
    negmax = small.tile([TD, 1], F32, tag="negmax")
    nc.vector.tensor_reduce(negmax, score_ps, mybir.AxisListType.X,
                            mybir.AluOpType.max, negate=True)
    E = singles.tile([TD, TE], F32)
    nc.scalar.activation(E, score_ps, AF.Exp, bias=negmax, scale=1.0)
    ssum = small.tile([TD, 1], F32, tag="ssum")
    nc.vector.tensor_reduce(ssum, E, mybir.AxisListType.X, mybir.AluOpType.add)
    rsum = small.tile([TD, 1], F32, tag="rsum")
    nc.vector.reciprocal(rsum, ssum)

    # ---- context = (E @ enc) * rsum ----
    ET = singles.tile([P, NS, TD], F32)
    for k in range(NS):
        pst = psum_tr.tile([P, TD], F32, tag="tr")
        nc.tensor.transpose(pst, E[:, k * P:(k + 1) * P], ident[:TD, :TD])
        nc.vector.tensor_copy(ET[:, k], pst)
    ctx_ps = psum_mm.tile([TD, D], F32, tag="mm")
    for k in range(NS):
        nc.tensor.matmul(ctx_ps, ET[:, k], enc_sb[:, k],
                         start=(k == 0), stop=(k == NS - 1))
    out_sb = singles.tile([TD, D], F32)
    nc.vector.tensor_scalar_mul(out_sb, ctx_ps, rsum)
    nc.sync.dma_start(out, out_sb)


_CACHE = {}


def _get_nc(reps=1):
    if ("nc", reps) in _CACHE:
        return _CACHE[("nc", reps)]
    nc = bacc.Bacc("TRN2", target_bir_lowering=False, debug=False,
                   enable_asserts=True, num_devices=B)
    ins = {
        "enc": nc.dram_tensor("enc", [TE, D], F32, kind="ExternalInput").ap(),
        "dec": nc.dram_tensor("dec", [TD, D], F32, kind="ExternalInput").ap(),
        "W1": nc.dram_tensor("W1", [D, U], F32, kind="ExternalInput").ap(),
        "b1": nc.dram_tensor("b1", [U, 1], F32, kind="ExternalInput").ap(),
        "W2": nc.dram_tensor("W2", [D, U], F32, kind="ExternalInput").ap(),
        "b2": nc.dram_tensor("b2", [U, 1], F32, kind="ExternalInput").ap(),
        "V": nc.dram_tensor("V", [U, 1], F32, kind="ExternalInput").ap(),
    }
    outs = {"out": nc.dram_tensor("out", [TD, D], F32, kind="ExternalOutput").ap()}
    from contextlib import ExitStack
    with tile.TileContext(nc) as tc:
        with ExitStack() as es:
            pools = _make_pools(es, tc)
            for _ in range(reps):
                _build_kernel(tc, pools, ins, outs)
    nc.compile()
    _CACHE[("nc", reps)] = nc
    return nc


def _in_maps(decoder_output, encoder_output, W1, b1, W2, b2, V):
    f = np.float32
    maps = []
    for b in range(B):
        maps.append({
            "enc": np.ascontiguousarray(encoder_output[b], dtype=f),
            "dec": np.ascontiguousarray(decoder_output[b], dtype=f),
            "W1": np.ascontiguousarray(W1, dtype=f),
            "b1": np.ascontiguousarray(np.asarray(b1, dtype=f).reshape(U, 1)),
            "W2": np.ascontiguousarray(W2, dtype=f),
            "b2": np.ascontiguousarray(np.asarray(b2, dtype=f).reshape(U, 1)),
            "V": np.ascontiguousarray(np.asarray(V, dtype=f).reshape(U, 1)),
        })
    return maps


def run(decoder_output, encoder_output, W1, b1, W2, b2, V, bV=None, *,
        trace=False, **trace_kwargs):
    nc = _get_nc()
    maps = _in_maps(decoder_output, encoder_output, W1, b1, W2, b2, V)
    res = bass_utils.run_bass_kernel_spmd(
        nc, maps, core_ids=list(range(B)), trace=trace, **trace_kwargs)
    out = np.stack([r["out"] for r in res.results], axis=0)
    return out.astype(np.float32), res


def kernel(decoder_output, encoder_output, W1, b1, W2, b2, V, bV=None):
    out, _ = run(decoder_output, encoder_output, W1, b1, W2, b2, V, bV)
    return out
